# revision 1
# baseline (speedup 1.0000x reference)
import sys

if "/opt/trn_rl_repo" not in sys.path:
    sys.path.insert(0, "/opt/trn_rl_repo")

import numpy as np

from concourse import bacc, mybir, tile
from concourse.bass_utils import run_bass_kernel_spmd

N_CORES = 8
B, C, H, W = 4096, 2, 64, 64
BPC = B // N_CORES          # 512 batches per core
NS = BPC // 16              # 32 supertiles of 16 maps each
NCHUNK = 8                  # data-loss chunks of [128, 4096] per tensor
CHUNK_F = 4096
GRID_D = 1.0 / (H - 1)
CLAMP_NEG_MIN = 27.6310211159  # -CLAMP_MIN

F32 = mybir.dt.float32
BF16 = mybir.dt.bfloat16


def _d1_matrix(n, d):
    m = np.zeros((n, n), dtype=np.float64)
    for i in range(1, n - 1):
        m[i, i - 1], m[i, i + 1] = -1.0, 1.0
    m[0, 0], m[0, 1], m[0, 2] = -3.0, 4.0, -1.0
    m[-1, -1], m[-1, -2], m[-1, -3] = 3.0, -4.0, 1.0
    return m / (2.0 * d)


def _d2_matrix(n, d):
    m = np.zeros((n, n), dtype=np.float64)
    for i in range(1, n - 1):
        m[i, i - 1], m[i, i], m[i, i + 1] = 1.0, -2.0, 1.0
    m[0, 0:4] = [2.0, -5.0, 4.0, -1.0]
    m[-1, -1], m[-1, -2], m[-1, -3], m[-1, -4] = 2.0, -5.0, 4.0, -1.0
    return m / (d * d)


def _build_consts():
    d1 = _d1_matrix(H, GRID_D)
    d2 = _d2_matrix(H, GRID_D)
    e = -(d2 + d1.T @ d1)            # sum(perm*(E@p)) == -sum(perm*d2p) - sum(d1perm*d1p)
    g = d1[H - 1, :] - d1[0, :]      # Neumann-boundary row functional

    import ml_dtypes

    # lhsT for the E matmul: out = lhsT.T @ rhs must be blkdiag(E, E) @ rhs
    c_e = np.zeros((128, 128), dtype=ml_dtypes.bfloat16)
    c_e[0:64, 0:64] = e.T.astype(ml_dtypes.bfloat16)
    c_e[64:128, 64:128] = e.T.astype(ml_dtypes.bfloat16)

    c_i = np.eye(128, dtype=ml_dtypes.bfloat16)

    # Banded reduction weights: slicing cols [63-2s : 127-2s] of this gives a
    # [128, 64] lhsT whose only nonzero columns are 2s (partitions 0:64) and
    # 2s+1 (partitions 64:128) — so supertile s's partition-sums land in PSUM
    # rows 2s, 2s+1 while start=False accumulation leaves other rows untouched.
    # bf16: ones are exact, and bf16 matmuls stream 4x faster than fp32.
    c_ones = np.zeros((128, 128), dtype=ml_dtypes.bfloat16)
    for p in range(128):
        c_ones[p, 63 + p // 64] = 1.0

    # Boundary fold: sum((perm + a 1^T) (.) E p) = sum(perm (.) E p) + g^T rowsums(p)
    # when E^T a = g, so the Neumann boundary terms ride the same product/reduce.
    a = np.linalg.lstsq(e.T, g, rcond=None)[0]
    assert np.abs(e.T @ a - g).max() < 1e-9
    c_a = np.zeros((128, 2), dtype=np.float32)
    c_a[:, 0] = np.tile(a, 2).astype(np.float32)
    c_a[:, 1] = -c_a[:, 0]

    return {"cE": c_e, "cI": c_i, "cOnes": c_ones, "cA": c_a}


def _build_nc():
    nc = bacc.Bacc("TRN2", target_bir_lowering=False, debug=False)

    x0 = nc.dram_tensor("x0", [NS, 2, 128, 512], BF16, kind="ExternalInput")
    mo = nc.dram_tensor("mo", [NCHUNK, 128, CHUNK_F], BF16, kind="ExternalInput")
    tg = nc.dram_tensor("tg", [NCHUNK, 128, CHUNK_F], BF16, kind="ExternalInput")
    c_e = nc.dram_tensor("cE", [128, 128], BF16, kind="ExternalInput")
    c_i = nc.dram_tensor("cI", [128, 128], BF16, kind="ExternalInput")
    c_ones = nc.dram_tensor(
        "cOnes", [128, 128], mybir.dt.bfloat16, kind="ExternalInput"
    )
    c_a = nc.dram_tensor("cA", [128, 2], F32, kind="ExternalInput")

    s1_out = nc.dram_tensor("s1", [64, 8], F32, kind="ExternalOutput")
    s2_out = nc.dram_tensor("s2", [64, 8], F32, kind="ExternalOutput")
    dstat_out = nc.dram_tensor("dstat", [128, NCHUNK], F32, kind="ExternalOutput")

    with tile.TileContext(nc) as tc:
        with (
            tc.tile_pool(name="consts", bufs=1) as cpool,
            tc.tile_pool(name="inp", bufs=4) as ipool,
            tc.tile_pool(name="work", bufs=2) as wpool,
            tc.tile_pool(name="dchunk", bufs=4) as dpool,
            tc.tile_pool(name="stats", bufs=1) as stpool,
            tc.tile_pool(name="pwork", bufs=2, space="PSUM") as pwpool,
            tc.tile_pool(name="ptrans", bufs=2, space="PSUM") as ptpool,
            tc.tile_pool(name="paccum", bufs=1, space="PSUM") as papool,
        ):
            ce = cpool.tile([128, 128], BF16, tag="ce")
            ci = cpool.tile([128, 128], BF16, tag="ci")
            cones = cpool.tile([128, 128], BF16, tag="cones")
            ca = cpool.tile([128, 2], F32, tag="ca")
            nc.sync.dma_start(ce[:], c_e[:])
            nc.sync.dma_start(ci[:], c_i[:])
            nc.sync.dma_start(cones[:], c_ones[:])
            nc.sync.dma_start(ca[:], c_a[:])

            sall = papool.tile([64, 512], F32, tag="sall")
            st = papool.tile([64, 512], F32, tag="st")
            dstat = stpool.tile([128, NCHUNK], F32, tag="dstat")

            for s in range(NS):
                p_t = ipool.tile([128, 512], BF16, tag="p")
                perm_t = ipool.tile([128, 512], BF16, tag="perm")
                # supertile layout: partition 64*r + h, free 64*j + w holds
                # batch 16*s + 8*r + j (channel 0 -> p_t, channel 1 -> perm_t)
                nc.sync.dma_start(p_t[:], x0[s, 0])
                nc.sync.dma_start(perm_t[:], x0[s, 1])

                tp = ptpool.tile([128, 1024], BF16, tag="tp")
                for k in range(4):
                    nc.tensor.transpose(
                        tp[:, 128 * k : 128 * (k + 1)],
                        p_t[:, 128 * k : 128 * (k + 1)],
                        ci[:],
                    )
                    nc.tensor.transpose(
                        tp[:, 512 + 128 * k : 512 + 128 * (k + 1)],
                        perm_t[:, 128 * k : 128 * (k + 1)],
                        ci[:],
                    )
                ts_s = wpool.tile([128, 1024], BF16, tag="ts")
                nc.scalar.copy(ts_s[:], tp[:])
                pt_s = ts_s[:, 0:512]
                permt_s = ts_s[:, 512:1024]

                ep = pwpool.tile([128, 512], F32, tag="ep")
                ept = pwpool.tile([128, 512], F32, tag="ept")
                nc.tensor.matmul(ep[:], ce[:], p_t[:], start=True, stop=True)
                nc.tensor.matmul(ept[:], ce[:], pt_s, start=True, stop=True)

                u1 = wpool.tile([128, 512], BF16, tag="u1")
                u2 = wpool.tile([128, 512], BF16, tag="u2")
                nc.vector.scalar_tensor_tensor(
                    u1[:], perm_t[:], ca[:, 0:1], ep[:],
                    op0=mybir.AluOpType.add, op1=mybir.AluOpType.mult,
                )
                nc.vector.scalar_tensor_tensor(
                    u2[:], permt_s, ca[:, 1:2], ept[:],
                    op0=mybir.AluOpType.add, op1=mybir.AluOpType.mult,
                )

                # per-(map,col) partition sums accumulated into persistent PSUM
                # rows 2s, 2s+1 via the banded lhsT slice
                lo, hi = 63 - 2 * s, 127 - 2 * s
                first, last = s == 0, s == NS - 1
                nc.tensor.matmul(
                    sall[:], cones[:, lo:hi], u1[:],
                    start=first, stop=last, skip_group_check=True,
                )
                nc.tensor.matmul(
                    st[:], cones[:, lo:hi], u2[:],
                    start=first, stop=last, skip_group_check=True,
                )

                # data loss: one [128, 4096] chunk every 4th supertile;
                # subtract alternates DVE/GpSimd to balance engine load
                if s % 4 == 2:
                    k = s // 4
                    mt = dpool.tile([128, CHUNK_F], BF16, tag="mt")
                    tt = dpool.tile([128, CHUNK_F], BF16, tag="tt")
                    nc.sync.dma_start(mt[:], mo[k])
                    nc.sync.dma_start(tt[:], tg[k])
                    eng = nc.vector if k % 2 == 0 else nc.gpsimd
                    eng.tensor_sub(mt[:], mt[:], tt[:])
                    nc.scalar.activation(
                        mt[:],
                        mt[:],
                        mybir.ActivationFunctionType.Square,
                        accum_out=dstat[:, k : k + 1],
                    )

            s1_t = stpool.tile([64, 8], F32, tag="s1t")
            s2_t = stpool.tile([64, 8], F32, tag="s2t")
            nc.vector.reduce_sum(
                s1_t[:],
                sall[:].rearrange("p (j w) -> p j w", j=8),
                axis=mybir.AxisListType.X,
            )
            nc.vector.reduce_sum(
                s2_t[:],
                st[:].rearrange("p (j w) -> p j w", j=8),
                axis=mybir.AxisListType.X,
            )
            nc.sync.dma_start(s1_out[:], s1_t[:])
            nc.sync.dma_start(s2_out[:], s2_t[:])
            nc.sync.dma_start(dstat_out[:], dstat[:])

    nc.compile()
    return nc


_NC = None
_CONSTS = None
LAST_RESULTS = None


def kernel(model_out, target, x0_hat, var, _trace=False, _trace_kwargs=None):
    global _NC, _CONSTS, LAST_RESULTS
    if _NC is None:
        _CONSTS = _build_consts()
        _NC = _build_nc()

    import ml_dtypes

    bf = ml_dtypes.bfloat16
    model_out = np.asarray(model_out).astype(bf)
    target = np.asarray(target).astype(bf)
    x0_hat = np.asarray(x0_hat, dtype=np.float32)
    var = np.asarray(var, dtype=np.float32)

    in_maps = []
    for c in range(N_CORES):
        lo, hi = c * BPC, (c + 1) * BPC
        # pre-arrange x0 into supertile layout so the device DMA reads are
        # contiguous: out[s, ch, 64r+h, 64j+w] = x0[lo + 16s+8r+j, ch, h, w]
        x0_arr = (
            x0_hat[lo:hi]
            .reshape(NS, 2, 8, 2, H, W)
            .transpose(0, 3, 1, 4, 2, 5)
            .astype(bf)
            .reshape(NS, 2, 128, 512)
        )
        in_maps.append(
            {
                "x0": x0_arr,
                "mo": model_out[lo:hi].reshape(NCHUNK, 128, CHUNK_F),
                "tg": target[lo:hi].reshape(NCHUNK, 128, CHUNK_F),
                **_CONSTS,
            }
        )

    kwargs = {}
    if _trace:
        kwargs["trace"] = True
        if _trace_kwargs:
            kwargs.update(_trace_kwargs)
    res = run_bass_kernel_spmd(_NC, in_maps, list(range(N_CORES)), **kwargs)
    LAST_RESULTS = res

    data_sum = 0.0
    nll_sum = 0.0
    for c in range(N_CORES):
        out = res.results[c]
        s1 = out["s1"].astype(np.float64)       # [64, 8]
        s2 = out["s2"].astype(np.float64)       # [64, 8]
        dstat = out["dstat"].astype(np.float64)  # [128, 16]

        # s1[2s+r, j] -> batch 16s + 8r + j
        r1 = s1.reshape(NS, 2, 8).reshape(BPC)
        # s2[2s+x, 2k+y] -> batch 16s + 8y + 2k + x
        r2 = s2.reshape(NS, 2, 4, 2).transpose(0, 3, 2, 1).reshape(BPC)
        r = (r1 + r2) / (H * W * 3.0)

        v = var[c * BPC : (c + 1) * BPC].astype(np.float64)
        nll = np.minimum(0.5 * r * r / v, CLAMP_NEG_MIN)
        nll_sum += nll.sum()
        data_sum += dstat.sum()

    loss = data_sum / (B * C * H * W) + nll_sum / B
    return np.float32(loss)



# revision 2
# speedup vs baseline: 1.3796x; 1.3796x over previous
import sys

if "/opt/trn_rl_repo" not in sys.path:
    sys.path.insert(0, "/opt/trn_rl_repo")

import numpy as np

from concourse import bacc, mybir, tile
from concourse.bass_utils import run_bass_kernel_spmd

N_CORES = 8
B, C, H, W = 4096, 2, 64, 64
BPC = B // N_CORES          # 512 batches per core
NS = BPC // 16              # 32 supertiles of 16 maps each
NXCH = 4                    # x0 chunks of [128, 4096] per plane (8 supertiles)
NCHUNK = 8                  # data-loss chunks of [128, 4096] per tensor
CHUNK_F = 4096
GRID_D = 1.0 / (H - 1)
CLAMP_NEG_MIN = 27.6310211159  # -CLAMP_MIN
ESCALE = 512.0              # E is sent as fp8(E / ESCALE)

F32 = mybir.dt.float32
BF16 = mybir.dt.bfloat16
FP8 = mybir.dt.float8e4


def _d1_matrix(n, d):
    m = np.zeros((n, n), dtype=np.float64)
    for i in range(1, n - 1):
        m[i, i - 1], m[i, i + 1] = -1.0, 1.0
    m[0, 0], m[0, 1], m[0, 2] = -3.0, 4.0, -1.0
    m[-1, -1], m[-1, -2], m[-1, -3] = 3.0, -4.0, 1.0
    return m / (2.0 * d)


def _d2_matrix(n, d):
    m = np.zeros((n, n), dtype=np.float64)
    for i in range(1, n - 1):
        m[i, i - 1], m[i, i], m[i, i + 1] = 1.0, -2.0, 1.0
    m[0, 0:4] = [2.0, -5.0, 4.0, -1.0]
    m[-1, -1], m[-1, -2], m[-1, -3], m[-1, -4] = 2.0, -5.0, 4.0, -1.0
    return m / (d * d)


def _build_consts():
    import ml_dtypes

    f8 = ml_dtypes.float8_e4m3

    d1 = _d1_matrix(H, GRID_D)
    d2 = _d2_matrix(H, GRID_D)
    e = -(d2 + d1.T @ d1)            # sum(perm*(E@p)) == -sum(perm*d2p) - sum(d1perm*d1p)
    g = d1[H - 1, :] - d1[0, :]      # Neumann-boundary row functional

    # E is shipped scaled into fp8; solve the boundary fold against the
    # QUANTIZED matrix so sum((perm + a 1^T) (.) Eq p) = sum(perm (.) Eq p)
    # + (g/ESCALE)^T rowsums(p) holds exactly for the matrix the PE uses.
    eq8 = (e / ESCALE).astype(f8)
    eq = eq8.astype(np.float64) * ESCALE
    a = np.linalg.lstsq(eq.T, g, rcond=None)[0]

    # lhsT for the E matmul: out = lhsT.T @ rhs must be blkdiag(Eq, Eq) @ rhs
    c_e = np.zeros((128, 128), dtype=f8)
    c_e[0:64, 0:64] = eq8.T
    c_e[64:128, 64:128] = eq8.T

    # Banded reduction weights: slicing cols [63-2s : 127-2s] of this gives a
    # [128, 64] lhsT whose only nonzero columns are 2s (partitions 0:64) and
    # 2s+1 (partitions 64:128) — so supertile s's partition-sums land in PSUM
    # rows 2s, 2s+1 while start=False accumulation leaves other rows untouched.
    c_ones = np.zeros((128, 128), dtype=ml_dtypes.bfloat16)
    for p in range(128):
        c_ones[p, 63 + p // 64] = 1.0

    # a-fold columns: +a for the h-direction plane, -a for the w-direction
    # plane (Neumann bc channel signs).
    c_a = np.zeros((128, 2), dtype=np.float32)
    c_a[:, 0] = np.tile(a, 2).astype(np.float32)
    c_a[:, 1] = -c_a[:, 0]

    return {"cE": c_e, "cOnes": c_ones, "cA": c_a}


def _build_nc():
    nc = bacc.Bacc("TRN2", target_bir_lowering=False, debug=False)

    # x0 planes in supertile layout, fp8. xh = h-major (partition 64r+h,
    # free 512s+64j+w), xt = per-map transposed (partition 64r+w, free
    # 512s+64j+h). Chunk c holds supertiles 8c..8c+7; plane 0 = p, 1 = perm.
    xh = nc.dram_tensor("xh", [NXCH, 2, 128, CHUNK_F], FP8, kind="ExternalInput")
    xt = nc.dram_tensor("xt", [NXCH, 2, 128, CHUNK_F], FP8, kind="ExternalInput")
    mo = nc.dram_tensor("mo", [NCHUNK, 128, CHUNK_F], FP8, kind="ExternalInput")
    tg = nc.dram_tensor("tg", [NCHUNK, 128, CHUNK_F], FP8, kind="ExternalInput")
    c_e = nc.dram_tensor("cE", [128, 128], FP8, kind="ExternalInput")
    c_ones = nc.dram_tensor("cOnes", [128, 128], BF16, kind="ExternalInput")
    c_a = nc.dram_tensor("cA", [128, 2], F32, kind="ExternalInput")

    s_out = nc.dram_tensor("s", [64, 8], F32, kind="ExternalOutput")
    dstat_out = nc.dram_tensor("dstat", [128, NCHUNK], F32, kind="ExternalOutput")

    with tile.TileContext(nc) as tc:
        with (
            tc.tile_pool(name="consts", bufs=1) as cpool,
            tc.tile_pool(name="inp", bufs=6) as ipool,
            tc.tile_pool(name="dchunk", bufs=6) as dpool,
            tc.tile_pool(name="diff", bufs=3) as dfpool,
            tc.tile_pool(name="work", bufs=4) as wpool,
            tc.tile_pool(name="stats", bufs=1) as stpool,
            tc.tile_pool(name="pwork", bufs=4, space="PSUM") as pwpool,
            tc.tile_pool(name="paccum", bufs=1, space="PSUM") as papool,
        ):
            ce = cpool.tile([128, 128], FP8, tag="ce")
            cones = cpool.tile([128, 128], BF16, tag="cones")
            ca = cpool.tile([128, 2], F32, tag="ca")
            nc.sync.dma_start(ce[:], c_e[:])
            nc.sync.dma_start(cones[:], c_ones[:])
            nc.sync.dma_start(ca[:], c_a[:])

            sall = papool.tile([64, 512], F32, tag="sall")
            dstat = stpool.tile([128, NCHUNK], F32, tag="dstat")

            for k in range(NCHUNK):
                plane = k % 2          # 0: h-major (xh), 1: w-major (xt)
                c = k // 2             # x0 chunk index
                src = xh if plane == 0 else xt

                p_t = ipool.tile([128, CHUNK_F], FP8, tag="p")
                perm_t = ipool.tile([128, CHUNK_F], FP8, tag="perm")
                nc.sync.dma_start(p_t[:], src[c, 0])
                nc.sync.dma_start(perm_t[:], src[c, 1])

                mt = dpool.tile([128, CHUNK_F], FP8, tag="mt")
                tt = dpool.tile([128, CHUNK_F], FP8, tag="tt")
                nc.sync.dma_start(mt[:], mo[k])
                nc.sync.dma_start(tt[:], tg[k])

                # data loss: diff on gpsimd, square+accum on scalar
                df = dfpool.tile([128, CHUNK_F], BF16, tag="df")
                nc.gpsimd.tensor_sub(df[:], mt[:], tt[:])
                nc.scalar.activation(
                    df[:],
                    df[:],
                    mybir.ActivationFunctionType.Square,
                    accum_out=dstat[:, k : k + 1],
                )

                # residual: 8 supertiles per chunk; both planes accumulate
                # into the same PSUM rows (identical batch mapping).
                for t in range(8):
                    s = 8 * c + t
                    ep = pwpool.tile([128, 512], F32, tag="ep")
                    nc.tensor.matmul(
                        ep[:],
                        ce[:],
                        p_t[:, 512 * t : 512 * (t + 1)],
                        start=True,
                        stop=True,
                    )
                    u = wpool.tile([128, 512], BF16, tag="u")
                    nc.vector.scalar_tensor_tensor(
                        u[:],
                        perm_t[:, 512 * t : 512 * (t + 1)],
                        ca[:, plane : plane + 1],
                        ep[:],
                        op0=mybir.AluOpType.add,
                        op1=mybir.AluOpType.mult,
                    )
                    lo, hi = 63 - 2 * s, 127 - 2 * s
                    first = k == 0 and t == 0
                    last = k == NCHUNK - 1 and t == 7
                    nc.tensor.matmul(
                        sall[:],
                        cones[:, lo:hi],
                        u[:],
                        start=first,
                        stop=last,
                        skip_group_check=True,
                    )

            s_t = stpool.tile([64, 8], F32, tag="st")
            nc.vector.reduce_sum(
                s_t[:],
                sall[:].rearrange("p (j w) -> p j w", j=8),
                axis=mybir.AxisListType.X,
            )
            nc.sync.dma_start(s_out[:], s_t[:])
            nc.sync.dma_start(dstat_out[:], dstat[:])

    nc.compile()
    return nc


_NC = None
_CONSTS = None
LAST_RESULTS = None


def kernel(model_out, target, x0_hat, var, _trace=False, _trace_kwargs=None):
    global _NC, _CONSTS, LAST_RESULTS
    if _NC is None:
        _CONSTS = _build_consts()
        _NC = _build_nc()

    import ml_dtypes

    f8 = ml_dtypes.float8_e4m3
    model_out = np.asarray(model_out).astype(f8)
    target = np.asarray(target).astype(f8)
    x0_hat = np.asarray(x0_hat).astype(f8)
    var = np.asarray(var, dtype=np.float32)

    in_maps = []
    for cid in range(N_CORES):
        lo, hi = cid * BPC, (cid + 1) * BPC
        # (s, r, j, ch, h, w) with batch = 16s + 8r + j
        x0c = x0_hat[lo:hi].reshape(NS, 2, 8, 2, H, W)
        # h-major: [ch, 64r+h, 512s+64j+w] -> chunks [4, 2, 128, 4096]
        xh_arr = (
            x0c.transpose(3, 1, 4, 0, 2, 5)
            .reshape(2, 128, NXCH, CHUNK_F)
            .transpose(2, 0, 1, 3)
            .copy()
        )
        # w-major (per-map transpose): [ch, 64r+w, 512s+64j+h]
        xt_arr = (
            x0c.transpose(3, 1, 5, 0, 2, 4)
            .reshape(2, 128, NXCH, CHUNK_F)
            .transpose(2, 0, 1, 3)
            .copy()
        )
        in_maps.append(
            {
                "xh": xh_arr,
                "xt": xt_arr,
                "mo": model_out[lo:hi].reshape(NCHUNK, 128, CHUNK_F),
                "tg": target[lo:hi].reshape(NCHUNK, 128, CHUNK_F),
                **_CONSTS,
            }
        )

    kwargs = {}
    if _trace:
        kwargs["trace"] = True
        if _trace_kwargs:
            kwargs.update(_trace_kwargs)
    res = run_bass_kernel_spmd(_NC, in_maps, list(range(N_CORES)), **kwargs)
    LAST_RESULTS = res

    data_sum = 0.0
    nll_sum = 0.0
    for cid in range(N_CORES):
        out = res.results[cid]
        s1 = out["s"].astype(np.float64)         # [64, 8]
        dstat = out["dstat"].astype(np.float64)  # [128, 8]

        # s1[2s+r, j] -> batch 16s + 8r + j
        r = s1.reshape(NS, 2, 8).reshape(BPC) * (ESCALE / (H * W * 3.0))

        v = var[cid * BPC : (cid + 1) * BPC].astype(np.float64)
        nll = np.minimum(0.5 * r * r / v, CLAMP_NEG_MIN)
        nll_sum += nll.sum()
        data_sum += dstat.sum()

    loss = data_sum / (B * C * H * W) + nll_sum / B
    return np.float32(loss)


# revision 4
# speedup vs baseline: 1.4483x; 1.0497x over previous
import sys

if "/opt/trn_rl_repo" not in sys.path:
    sys.path.insert(0, "/opt/trn_rl_repo")

import numpy as np

from concourse import bacc, mybir, tile
from concourse.bass_utils import run_bass_kernel_spmd

N_CORES = 8
B, C, H, W = 4096, 2, 64, 64
BPC = B // N_CORES          # 512 batches per core
NS = BPC // 16              # 32 supertiles of 16 maps each
NXCH = 4                    # x0 chunks of [128, 4096] per plane (8 supertiles)
NCHUNK = 8                  # data-loss chunks of [128, 4096] per tensor
CHUNK_F = 4096
GRID_D = 1.0 / (H - 1)
CLAMP_NEG_MIN = 27.6310211159  # -CLAMP_MIN
ESCALE = 512.0              # E is sent as fp8(E / ESCALE)

# Engine for each data-loss chunk's subtract: "dve" | "gps" | "dr"
# ("dr" = DoubleRow matmul subtract on the tensor engine, squares from PSUM)
SUB_ENGINE = ["dve", "dve", "dve", "gps", "gps", "gps", "gps", "gps"]

F32 = mybir.dt.float32
BF16 = mybir.dt.bfloat16
FP8 = mybir.dt.float8e4


def _d1_matrix(n, d):
    m = np.zeros((n, n), dtype=np.float64)
    for i in range(1, n - 1):
        m[i, i - 1], m[i, i + 1] = -1.0, 1.0
    m[0, 0], m[0, 1], m[0, 2] = -3.0, 4.0, -1.0
    m[-1, -1], m[-1, -2], m[-1, -3] = 3.0, -4.0, 1.0
    return m / (2.0 * d)


def _d2_matrix(n, d):
    m = np.zeros((n, n), dtype=np.float64)
    for i in range(1, n - 1):
        m[i, i - 1], m[i, i], m[i, i + 1] = 1.0, -2.0, 1.0
    m[0, 0:4] = [2.0, -5.0, 4.0, -1.0]
    m[-1, -1], m[-1, -2], m[-1, -3], m[-1, -4] = 2.0, -5.0, 4.0, -1.0
    return m / (d * d)


def _build_consts():
    import ml_dtypes

    f8 = ml_dtypes.float8_e4m3
    bf = ml_dtypes.bfloat16

    d1 = _d1_matrix(H, GRID_D)
    d2 = _d2_matrix(H, GRID_D)
    e = -(d2 + d1.T @ d1)            # sum(perm*(E@p)) == -sum(perm*d2p) - sum(d1perm*d1p)
    g = d1[H - 1, :] - d1[0, :]      # Neumann-boundary row functional

    # E ships scaled into fp8; solve the boundary fold against the QUANTIZED
    # matrix so sum((perm + a 1^T) (.) Eq p) = sum(perm (.) Eq p)
    # + (g/ESCALE)^T rowsums(p) holds exactly for the matrix the PE uses.
    eq8 = (e / ESCALE).astype(f8)
    eq = eq8.astype(np.float64) * ESCALE
    a = np.linalg.lstsq(eq.T, g, rcond=None)[0]

    # lhsT for the E matmul: out = lhsT.T @ rhs must be blkdiag(Eq, Eq) @ rhs
    c_e = np.zeros((128, 128), dtype=f8)
    c_e[0:64, 0:64] = eq8.T
    c_e[64:128, 64:128] = eq8.T

    # Banded reduction weights, padded to 128-column slices so LDWEIGHTS gets
    # fast-weight-load. Slice [62-2s : 190-2s] has ones only at within-slice
    # columns 2s (partitions 0:64) and 2s+1 (partitions 64:128), so supertile
    # s's partition sums land in PSUM rows 2s, 2s+1; the other 126 output rows
    # accumulate zeros.
    c_ones = np.zeros((128, 190), dtype=bf)
    for p in range(128):
        c_ones[p, 62 + p // 64] = 1.0

    # a-fold columns: +a for the h-direction plane, -a for the w-direction
    # plane (Neumann bc channel signs).
    c_a = np.zeros((128, 2), dtype=np.float32)
    c_a[:, 0] = np.tile(a, 2).astype(np.float32)
    c_a[:, 1] = -c_a[:, 0]

    # DoubleRow subtract weights: [128, 2, 128] flattened two-major so that
    # out = W0.T @ R0 + W1.T @ R1 = R0 - R1 with W0 = I, W1 = -I.
    c_w = np.zeros((128, 256), dtype=f8)
    c_w[:, 0:128] = np.eye(128, dtype=f8)
    c_w[:, 128:256] = -np.eye(128, dtype=f8)

    return {"cE": c_e, "cOnes": c_ones, "cA": c_a, "cW": c_w}


def _build_nc():
    nc = bacc.Bacc("TRN2", target_bir_lowering=False, debug=False)

    # x0 planes in supertile layout, fp8. xh = h-major (partition 64r+h,
    # free 512s+64j+w), xt = per-map transposed (partition 64r+w, free
    # 512s+64j+h). Chunk c holds supertiles 8c..8c+7; plane 0 = p, 1 = perm.
    xh = nc.dram_tensor("xh", [NXCH, 2, 128, CHUNK_F], FP8, kind="ExternalInput")
    xt = nc.dram_tensor("xt", [NXCH, 2, 128, CHUNK_F], FP8, kind="ExternalInput")
    # data-loss chunks: [mo | tg] concatenated along free
    md = nc.dram_tensor("md", [NCHUNK, 128, 2 * CHUNK_F], FP8, kind="ExternalInput")
    c_e = nc.dram_tensor("cE", [128, 128], FP8, kind="ExternalInput")
    c_ones = nc.dram_tensor("cOnes", [128, 190], BF16, kind="ExternalInput")
    c_a = nc.dram_tensor("cA", [128, 2], F32, kind="ExternalInput")
    c_w = nc.dram_tensor("cW", [128, 256], FP8, kind="ExternalInput")

    s_out = nc.dram_tensor("s", [64, 8], F32, kind="ExternalOutput")
    dstat_out = nc.dram_tensor("dstat", [128, 4 * NCHUNK], F32, kind="ExternalOutput")

    with tile.TileContext(nc) as tc:
        with (
            tc.tile_pool(name="consts", bufs=1) as cpool,
            tc.tile_pool(name="inp", bufs=6) as ipool,
            tc.tile_pool(name="dchunk", bufs=3) as dpool,
            tc.tile_pool(name="diff", bufs=3) as dfpool,
            tc.tile_pool(name="work", bufs=4) as wpool,
            tc.tile_pool(name="stats", bufs=1) as stpool,
            tc.tile_pool(name="pwork", bufs=3, space="PSUM") as pwpool,
            tc.tile_pool(name="paccum", bufs=1, space="PSUM") as papool,
        ):
            ce = cpool.tile([128, 128], FP8, tag="ce")
            cones = cpool.tile([128, 190], BF16, tag="cones")
            ca = cpool.tile([128, 2], F32, tag="ca")
            cw = cpool.tile([128, 256], FP8, tag="cw")
            nc.sync.dma_start(ce[:], c_e[:])
            nc.sync.dma_start(cones[:], c_ones[:])
            nc.sync.dma_start(ca[:], c_a[:])
            nc.sync.dma_start(cw[:], c_w[:])

            sall = papool.tile([128, 512], F32, tag="sall")
            dstat = stpool.tile([128, 4 * NCHUNK], F32, tag="dstat")

            for k in range(NCHUNK):
                plane = k % 2          # 0: h-major (xh), 1: w-major (xt)
                c = k // 2             # x0 chunk index
                src = xh if plane == 0 else xt

                p_t = ipool.tile([128, CHUNK_F], FP8, tag="p")
                perm_t = ipool.tile([128, CHUNK_F], FP8, tag="perm")
                nc.sync.dma_start(p_t[:], src[c, 0])
                nc.sync.dma_start(perm_t[:], src[c, 1])

                mt = dpool.tile([128, 2 * CHUNK_F], FP8, tag="mt")
                nc.sync.dma_start(mt[:], md[k])

                # data loss for this chunk
                sub = SUB_ENGINE[k]
                if sub == "dr":
                    # DoubleRow matmul subtract into PSUM; scalar squares from
                    # PSUM per 1024 columns.
                    for g in range(4):
                        dp = pwpool.tile([128, 1024], F32, tag="ep")
                        mt3 = mt[:].rearrange("p (two n) -> p two n", two=2)
                        for h2 in range(2):
                            t = 2 * g + h2
                            rhs3 = mt3[:, :, 512 * t : 512 * (t + 1)]
                            nc.tensor.matmul(
                                dp[:, 512 * h2 : 512 * (h2 + 1)],
                                cw[:].rearrange("p (two m) -> p two m", two=2),
                                rhs3,
                                start=True,
                                stop=True,
                                perf_mode=mybir.MatmulPerfMode.DoubleRow,
                            )
                        junk = dfpool.tile([128, 1024], BF16, tag="df")
                        nc.scalar.activation(
                            junk[:],
                            dp[:],
                            mybir.ActivationFunctionType.Square,
                            accum_out=dstat[:, 4 * k + g : 4 * k + g + 1],
                        )
                else:
                    df = dfpool.tile([128, CHUNK_F], BF16, tag="df")
                    eng = nc.vector if sub == "dve" else nc.gpsimd
                    eng.tensor_sub(
                        df[:], mt[:, 0:CHUNK_F], mt[:, CHUNK_F : 2 * CHUNK_F]
                    )
                    nc.scalar.activation(
                        df[:],
                        df[:],
                        mybir.ActivationFunctionType.Square,
                        accum_out=dstat[:, 4 * k : 4 * k + 1],
                    )

                # residual: 8 supertiles per chunk in pairs; both planes
                # accumulate into the same PSUM rows (identical batch mapping).
                for g in range(4):
                    ep = pwpool.tile([128, 1024], F32, tag="ep")
                    for h2 in range(2):
                        t = 2 * g + h2
                        nc.tensor.matmul(
                            ep[:, 512 * h2 : 512 * (h2 + 1)],
                            ce[:],
                            p_t[:, 512 * t : 512 * (t + 1)],
                            start=True,
                            stop=True,
                        )
                    u = wpool.tile([128, 1024], BF16, tag="u")
                    nc.vector.scalar_tensor_tensor(
                        u[:],
                        perm_t[:, 1024 * g : 1024 * (g + 1)],
                        ca[:, plane : plane + 1],
                        ep[:],
                        op0=mybir.AluOpType.add,
                        op1=mybir.AluOpType.mult,
                    )
                    for h2 in range(2):
                        s = 8 * c + 2 * g + h2
                        lo, hi = 62 - 2 * s, 190 - 2 * s
                        first = k == 0 and g == 0 and h2 == 0
                        last = k == NCHUNK - 1 and g == 3 and h2 == 1
                        nc.tensor.matmul(
                            sall[:],
                            cones[:, lo:hi],
                            u[:, 512 * h2 : 512 * (h2 + 1)],
                            start=first,
                            stop=last,
                            skip_group_check=True,
                        )

            s_t = stpool.tile([64, 8], F32, tag="st")
            nc.vector.reduce_sum(
                s_t[:],
                sall[0:64, :].rearrange("p (j w) -> p j w", j=8),
                axis=mybir.AxisListType.X,
            )
            nc.sync.dma_start(s_out[:], s_t[:])
            nc.sync.dma_start(dstat_out[:], dstat[:])

    nc.compile()
    return nc


_NC = None
_CONSTS = None
LAST_RESULTS = None


def kernel(model_out, target, x0_hat, var, _trace=False, _trace_kwargs=None):
    global _NC, _CONSTS, LAST_RESULTS
    if _NC is None:
        _CONSTS = _build_consts()
        _NC = _build_nc()

    import ml_dtypes

    f8 = ml_dtypes.float8_e4m3
    model_out = np.asarray(model_out).astype(f8)
    target = np.asarray(target).astype(f8)
    x0_hat = np.asarray(x0_hat).astype(f8)
    var = np.asarray(var, dtype=np.float32)

    in_maps = []
    for cid in range(N_CORES):
        lo, hi = cid * BPC, (cid + 1) * BPC
        # (s, r, j, ch, h, w) with batch = 16s + 8r + j
        x0c = x0_hat[lo:hi].reshape(NS, 2, 8, 2, H, W)
        # h-major: [ch, 64r+h, 512s+64j+w] -> chunks [4, 2, 128, 4096]
        xh_arr = (
            x0c.transpose(3, 1, 4, 0, 2, 5)
            .reshape(2, 128, NXCH, CHUNK_F)
            .transpose(2, 0, 1, 3)
            .copy()
        )
        # w-major (per-map transpose): [ch, 64r+w, 512s+64j+h]
        xt_arr = (
            x0c.transpose(3, 1, 5, 0, 2, 4)
            .reshape(2, 128, NXCH, CHUNK_F)
            .transpose(2, 0, 1, 3)
            .copy()
        )
        md_arr = np.concatenate(
            [
                model_out[lo:hi].reshape(NCHUNK, 128, CHUNK_F),
                target[lo:hi].reshape(NCHUNK, 128, CHUNK_F),
            ],
            axis=-1,
        )
        in_maps.append({"xh": xh_arr, "xt": xt_arr, "md": md_arr, **_CONSTS})

    kwargs = {}
    if _trace:
        kwargs["trace"] = True
        if _trace_kwargs:
            kwargs.update(_trace_kwargs)
    res = run_bass_kernel_spmd(_NC, in_maps, list(range(N_CORES)), **kwargs)
    LAST_RESULTS = res

    data_sum = 0.0
    nll_sum = 0.0
    for cid in range(N_CORES):
        out = res.results[cid]
        s1 = out["s"].astype(np.float64)         # [64, 8]
        dstat = out["dstat"].astype(np.float64)  # [128, 32]

        # s1[2s+r, j] -> batch 16s + 8r + j
        r = s1.reshape(NS, 2, 8).reshape(BPC) * (ESCALE / (H * W * 3.0))

        v = var[cid * BPC : (cid + 1) * BPC].astype(np.float64)
        nll = np.minimum(0.5 * r * r / v, CLAMP_NEG_MIN)
        nll_sum += nll.sum()
        data_sum += dstat.sum()

    loss = data_sum / (B * C * H * W) + nll_sum / B
    return np.float32(loss)


# revision 6
# speedup vs baseline: 1.4606x; 1.0085x over previous
import sys

if "/opt/trn_rl_repo" not in sys.path:
    sys.path.insert(0, "/opt/trn_rl_repo")

import numpy as np

from concourse import bacc, mybir, tile
from concourse.bass_utils import run_bass_kernel_spmd

N_CORES = 8
B, C, H, W = 4096, 2, 64, 64
BPC = B // N_CORES          # 512 batches per core
NS = BPC // 16              # 32 supertiles of 16 maps each
NXCH = 4                    # x0 chunks of [128, 4096] per plane (8 supertiles)
NCHUNK = 8                  # data-loss chunks of [128, 4096] per tensor
CHUNK_F = 4096
GRID_D = 1.0 / (H - 1)
CLAMP_NEG_MIN = 27.6310211159  # -CLAMP_MIN
ESCALE = 512.0              # E is sent as fp8(E / ESCALE)

# Engine for each data-loss chunk's subtract: "dve" | "gps" | "dr"
# ("dr" = DoubleRow matmul subtract on the tensor engine, squares from PSUM)
SUB_ENGINE = ["dr", "gps", "dr", "gps", "dr", "gps", "dr", "gps"]

F32 = mybir.dt.float32
BF16 = mybir.dt.bfloat16
FP8 = mybir.dt.float8e4


def _d1_matrix(n, d):
    m = np.zeros((n, n), dtype=np.float64)
    for i in range(1, n - 1):
        m[i, i - 1], m[i, i + 1] = -1.0, 1.0
    m[0, 0], m[0, 1], m[0, 2] = -3.0, 4.0, -1.0
    m[-1, -1], m[-1, -2], m[-1, -3] = 3.0, -4.0, 1.0
    return m / (2.0 * d)


def _d2_matrix(n, d):
    m = np.zeros((n, n), dtype=np.float64)
    for i in range(1, n - 1):
        m[i, i - 1], m[i, i], m[i, i + 1] = 1.0, -2.0, 1.0
    m[0, 0:4] = [2.0, -5.0, 4.0, -1.0]
    m[-1, -1], m[-1, -2], m[-1, -3], m[-1, -4] = 2.0, -5.0, 4.0, -1.0
    return m / (d * d)


def _build_consts():
    import ml_dtypes

    f8 = ml_dtypes.float8_e4m3
    bf = ml_dtypes.bfloat16

    d1 = _d1_matrix(H, GRID_D)
    d2 = _d2_matrix(H, GRID_D)
    e = -(d2 + d1.T @ d1)            # sum(perm*(E@p)) == -sum(perm*d2p) - sum(d1perm*d1p)
    g = d1[H - 1, :] - d1[0, :]      # Neumann-boundary row functional

    # E ships scaled into fp8; solve the boundary fold against the QUANTIZED
    # matrix so sum((perm + a 1^T) (.) Eq p) = sum(perm (.) Eq p)
    # + (g/ESCALE)^T rowsums(p) holds exactly for the matrix the PE uses.
    eq8 = (e / ESCALE).astype(f8)
    eq = eq8.astype(np.float64) * ESCALE
    a = np.linalg.lstsq(eq.T, g, rcond=None)[0]

    # lhsT for the E matmul: out = lhsT.T @ rhs must be blkdiag(Eq, Eq) @ rhs
    c_e = np.zeros((128, 128), dtype=f8)
    c_e[0:64, 0:64] = eq8.T
    c_e[64:128, 64:128] = eq8.T

    # Banded reduction weights, padded to 128-column slices so LDWEIGHTS gets
    # fast-weight-load. Slice [62-2s : 190-2s] has ones only at within-slice
    # columns 2s (partitions 0:64) and 2s+1 (partitions 64:128), so supertile
    # s's partition sums land in PSUM rows 2s, 2s+1; the other 126 output rows
    # accumulate zeros.
    c_ones = np.zeros((128, 190), dtype=bf)
    for p in range(128):
        c_ones[p, 62 + p // 64] = 1.0

    # a-fold columns: +a for the h-direction plane, -a for the w-direction
    # plane (Neumann bc channel signs).
    c_a = np.zeros((128, 2), dtype=np.float32)
    c_a[:, 0] = np.tile(a, 2).astype(np.float32)
    c_a[:, 1] = -c_a[:, 0]

    # DoubleRow subtract weights: [128, 2, 128] flattened two-major so that
    # out = W0.T @ R0 + W1.T @ R1 = R0 - R1 with W0 = I, W1 = -I.
    c_w = np.zeros((128, 256), dtype=f8)
    c_w[:, 0:128] = np.eye(128, dtype=f8)
    c_w[:, 128:256] = -np.eye(128, dtype=f8)

    return {"cE": c_e, "cOnes": c_ones, "cA": c_a, "cW": c_w}


def _build_nc():
    nc = bacc.Bacc("TRN2", target_bir_lowering=False, debug=False)

    # x0 planes in supertile layout, fp8. xh = h-major (partition 64r+h,
    # free 512s+64j+w), xt = per-map transposed (partition 64r+w, free
    # 512s+64j+h). Chunk c holds supertiles 8c..8c+7; plane 0 = p, 1 = perm.
    xh = nc.dram_tensor("xh", [NXCH, 2, 128, CHUNK_F], FP8, kind="ExternalInput")
    xt = nc.dram_tensor("xt", [NXCH, 2, 128, CHUNK_F], FP8, kind="ExternalInput")
    # data-loss chunks: [mo | tg] concatenated along free
    md = nc.dram_tensor("md", [NCHUNK, 128, 2 * CHUNK_F], FP8, kind="ExternalInput")
    c_e = nc.dram_tensor("cE", [128, 128], FP8, kind="ExternalInput")
    c_ones = nc.dram_tensor("cOnes", [128, 190], BF16, kind="ExternalInput")
    c_a = nc.dram_tensor("cA", [128, 2], F32, kind="ExternalInput")
    c_w = nc.dram_tensor("cW", [128, 256], FP8, kind="ExternalInput")

    s_out = nc.dram_tensor("s", [64, 8], F32, kind="ExternalOutput")
    dstat_out = nc.dram_tensor("dstat", [128, 4 * NCHUNK], F32, kind="ExternalOutput")

    with tile.TileContext(nc) as tc:
        with (
            tc.tile_pool(name="consts", bufs=1) as cpool,
            tc.tile_pool(name="inp", bufs=8) as ipool,
            tc.tile_pool(name="dchunk", bufs=5) as dpool,
            tc.tile_pool(name="diff", bufs=3) as dfpool,
            tc.tile_pool(name="work", bufs=4) as wpool,
            tc.tile_pool(name="stats", bufs=1) as stpool,
            tc.tile_pool(name="pwork", bufs=3, space="PSUM") as pwpool,
            tc.tile_pool(name="paccum", bufs=1, space="PSUM") as papool,
        ):
            ce = cpool.tile([128, 128], FP8, tag="ce")
            cones = cpool.tile([128, 190], BF16, tag="cones")
            ca = cpool.tile([128, 2], F32, tag="ca")
            cw = cpool.tile([128, 256], FP8, tag="cw")
            nc.sync.dma_start(ce[:], c_e[:])
            nc.sync.dma_start(cones[:], c_ones[:])
            nc.sync.dma_start(ca[:], c_a[:])
            nc.sync.dma_start(cw[:], c_w[:])

            sall = papool.tile([128, 512], F32, tag="sall")
            dstat = stpool.tile([128, 4 * NCHUNK], F32, tag="dstat")

            for k in range(NCHUNK):
                plane = k % 2          # 0: h-major (xh), 1: w-major (xt)
                c = k // 2             # x0 chunk index
                src = xh if plane == 0 else xt

                p_t = ipool.tile([128, CHUNK_F], FP8, tag="p")
                perm_t = ipool.tile([128, CHUNK_F], FP8, tag="perm")
                nc.sync.dma_start(p_t[:], src[c, 0])
                nc.sync.dma_start(perm_t[:], src[c, 1])

                mt = dpool.tile([128, 2 * CHUNK_F], FP8, tag="mt")
                nc.sync.dma_start(mt[:], md[k])

                # data loss for this chunk
                sub = SUB_ENGINE[k]
                if sub == "dr":
                    # DoubleRow matmul subtract into PSUM; scalar squares from
                    # PSUM per 1024 columns.
                    for g in range(4):
                        dp = pwpool.tile([128, 1024], F32, tag="ep")
                        mt3 = mt[:].rearrange("p (two n) -> p two n", two=2)
                        for h2 in range(2):
                            t = 2 * g + h2
                            rhs3 = mt3[:, :, 512 * t : 512 * (t + 1)]
                            nc.tensor.matmul(
                                dp[:, 512 * h2 : 512 * (h2 + 1)],
                                cw[:].rearrange("p (two m) -> p two m", two=2),
                                rhs3,
                                start=True,
                                stop=True,
                                perf_mode=mybir.MatmulPerfMode.DoubleRow,
                            )
                        junk = dfpool.tile([128, 1024], BF16, tag="df")
                        nc.scalar.activation(
                            junk[:],
                            dp[:],
                            mybir.ActivationFunctionType.Square,
                            accum_out=dstat[:, 4 * k + g : 4 * k + g + 1],
                        )
                else:
                    df = dfpool.tile([128, CHUNK_F], BF16, tag="df")
                    eng = nc.vector if sub == "dve" else nc.gpsimd
                    eng.tensor_sub(
                        df[:], mt[:, 0:CHUNK_F], mt[:, CHUNK_F : 2 * CHUNK_F]
                    )
                    nc.scalar.activation(
                        df[:],
                        df[:],
                        mybir.ActivationFunctionType.Square,
                        accum_out=dstat[:, 4 * k : 4 * k + 1],
                    )

                # residual: 8 supertiles per chunk in pairs; both planes
                # accumulate into the same PSUM rows (identical batch mapping).
                for g in range(4):
                    ep = pwpool.tile([128, 1024], F32, tag="ep")
                    for h2 in range(2):
                        t = 2 * g + h2
                        nc.tensor.matmul(
                            ep[:, 512 * h2 : 512 * (h2 + 1)],
                            ce[:],
                            p_t[:, 512 * t : 512 * (t + 1)],
                            start=True,
                            stop=True,
                        )
                    u = wpool.tile([128, 1024], BF16, tag="u")
                    nc.vector.scalar_tensor_tensor(
                        u[:],
                        perm_t[:, 1024 * g : 1024 * (g + 1)],
                        ca[:, plane : plane + 1],
                        ep[:],
                        op0=mybir.AluOpType.add,
                        op1=mybir.AluOpType.mult,
                    )
                    for h2 in range(2):
                        s = 8 * c + 2 * g + h2
                        lo, hi = 62 - 2 * s, 190 - 2 * s
                        first = k == 0 and g == 0 and h2 == 0
                        last = k == NCHUNK - 1 and g == 3 and h2 == 1
                        nc.tensor.matmul(
                            sall[:],
                            cones[:, lo:hi],
                            u[:, 512 * h2 : 512 * (h2 + 1)],
                            start=first,
                            stop=last,
                            skip_group_check=True,
                        )

            s_t = stpool.tile([64, 8], F32, tag="st")
            nc.vector.reduce_sum(
                s_t[:],
                sall[0:64, :].rearrange("p (j w) -> p j w", j=8),
                axis=mybir.AxisListType.X,
            )
            nc.sync.dma_start(s_out[:], s_t[:])
            nc.sync.dma_start(dstat_out[:], dstat[:])

    nc.compile()
    return nc


_NC = None
_CONSTS = None
LAST_RESULTS = None


def kernel(model_out, target, x0_hat, var, _trace=False, _trace_kwargs=None):
    global _NC, _CONSTS, LAST_RESULTS
    if _NC is None:
        _CONSTS = _build_consts()
        _NC = _build_nc()

    import ml_dtypes

    f8 = ml_dtypes.float8_e4m3
    model_out = np.asarray(model_out).astype(f8)
    target = np.asarray(target).astype(f8)
    x0_hat = np.asarray(x0_hat).astype(f8)
    var = np.asarray(var, dtype=np.float32)

    in_maps = []
    for cid in range(N_CORES):
        lo, hi = cid * BPC, (cid + 1) * BPC
        # (s, r, j, ch, h, w) with batch = 16s + 8r + j
        x0c = x0_hat[lo:hi].reshape(NS, 2, 8, 2, H, W)
        # h-major: [ch, 64r+h, 512s+64j+w] -> chunks [4, 2, 128, 4096]
        xh_arr = (
            x0c.transpose(3, 1, 4, 0, 2, 5)
            .reshape(2, 128, NXCH, CHUNK_F)
            .transpose(2, 0, 1, 3)
            .copy()
        )
        # w-major (per-map transpose): [ch, 64r+w, 512s+64j+h]
        xt_arr = (
            x0c.transpose(3, 1, 5, 0, 2, 4)
            .reshape(2, 128, NXCH, CHUNK_F)
            .transpose(2, 0, 1, 3)
            .copy()
        )
        md_arr = np.concatenate(
            [
                model_out[lo:hi].reshape(NCHUNK, 128, CHUNK_F),
                target[lo:hi].reshape(NCHUNK, 128, CHUNK_F),
            ],
            axis=-1,
        )
        in_maps.append({"xh": xh_arr, "xt": xt_arr, "md": md_arr, **_CONSTS})

    kwargs = {}
    if _trace:
        kwargs["trace"] = True
        if _trace_kwargs:
            kwargs.update(_trace_kwargs)
    res = run_bass_kernel_spmd(_NC, in_maps, list(range(N_CORES)), **kwargs)
    LAST_RESULTS = res

    data_sum = 0.0
    nll_sum = 0.0
    for cid in range(N_CORES):
        out = res.results[cid]
        s1 = out["s"].astype(np.float64)         # [64, 8]
        dstat = out["dstat"].astype(np.float64)  # [128, 32]

        # s1[2s+r, j] -> batch 16s + 8r + j
        r = s1.reshape(NS, 2, 8).reshape(BPC) * (ESCALE / (H * W * 3.0))

        v = var[cid * BPC : (cid + 1) * BPC].astype(np.float64)
        nll = np.minimum(0.5 * r * r / v, CLAMP_NEG_MIN)
        nll_sum += nll.sum()
        data_sum += dstat.sum()

    loss = data_sum / (B * C * H * W) + nll_sum / B
    return np.float32(loss)
